# revision 12
# baseline (speedup 1.0000x reference)
"""Trainium2 Bass kernel for nn_Correlation (stereo cost volume).

  out[b, d, h, w] = mean_c( x[b,c,h,w] * y[b,c,h,w-d] ),  w >= d else 0
  B=8, C=32, H=256, W=512, D=48  (maxdisp=48)

Sharding: data-parallel over batch B across the 8 NeuronCores (one batch
element per core).  Each core computes its full [D, H, W] cost volume.

Per-core algorithm (fp16 inputs, fp32 psum, fp16 output):
  - Host pre-casts x/y to fp16; x/y staged in FOUR 32-partition slabs
    (4 h-row groups at partitions 0/32/64/96); y contiguous per slab
    with 47-col lead + 48-col tail zeros.
  - Per (h, 128-col w-tile): FOUR M=32 col-tiled matmuls (c = 0..3):
      stationary x[w0+32c : w0+32c+32] -> psum parts 32c:32c+32
      moving    y[w0+32c-47 .. w0+32c+33)   (K=32, N=80)
    A full h-row's valid band is ONE dense [128, 320] psum bank (1.67x
    ideal output bytes).  The 64 matmuls of a 4-row group are emitted in
    a staggered order: consecutive matmuls differ in row-group (moving
    stream) and alternate 64-col array halves (psum drain port), and
    ps[0..3] complete at slots ~32/~52/~62/64 so psum->SBUF copies
    overlap the remaining matmuls instead of bunching at group end.
  - One DVE/ACT copy per h-row scales the [128,320] psum by 1/32 into a
    fp16 G tile [128, 4, 4, 320] (4 h-groups x 4 slab-rows), DMA'd
    straight to the DRAM output in band layout (10240B-contiguous
    descriptors).  No DRAM->DRAM skew pass, no scratch round trip.
  - Input loads for iteration it+1 are emitted before it's compute
    (bufs=3) so they stream during compute.
  - The host un-skews the diagonals with a strided numpy view, zeroes
    the w<d triangle, and assembles [B, D, H, W] fp32.
"""

import sys

sys.path.insert(0, "/opt/trn_rl_repo")

import numpy as np
from contextlib import ExitStack

import concourse.bass as bass
import concourse.tile as tile
from concourse import mybir
from concourse import bass_utils

B = 8
C = 32
H = 256
W = 512
D = 48
LEAD = D - 1            # 47 cols before row 0 of each slab
TAIL = 48               # cols after the last row (t=3,c=3 window overrun)
NW = W // 128           # 4 w-tiles per row
MMN = 80                # moving cols per col-tile matmul (32 + 47, padded)
GQ = NW * MMN           # 320 band cols per h-row
G = 16                  # h rows per slab per iteration
NSL = 4                 # slabs (row-groups) per iteration
GPACK = 4               # g-iterations packed per output DMA
NGQ = H // (NSL * GPACK)  # 16 output groups (16 h-rows each)
YCOLS = LEAD + G * W + TAIL


def _mm_slots():
    """64 (t, c, ss) slots per 4-row group: slabs 0/1 fill slots 0-31 and
    slabs 2/3 fill 32-63 (so ps[0]/ps[1] complete halfway and their copies
    overlap the remaining matmuls); consecutive slots differ in row-group
    (moving stream) AND 64-col array half (psum drain port)."""
    h0l = [(t, cc) for t in range(4) for cc in (0, 1)]       # array half 0
    h1l = [(t, 2 + cc) for t in range(4) for cc in (0, 1)]   # array half 1
    slots = []
    for a, b in ((0, 1), (2, 3)):
        for i in range(8):
            slots.append((*h0l[i], a))
            slots.append((*h1l[i], b))
        for i in range(8):
            slots.append((*h1l[i], a))
            slots.append((*h0l[i], b))
    return slots


MM_SLOTS = _mm_slots()
assert len(MM_SLOTS) == 64 and len(set(MM_SLOTS)) == 64


def _split_waits(nc, max_waits=1):
    """Walrus codegen accepts at most ONE sync wait per instruction; Tile
    attaches several.  Split extra waits onto preceding NoOps on the same
    engine queue (dispatch is in-order, waits gate dispatch)."""
    for fn in nc.m.functions:
        for blk in fn.blocks:
            newl = []
            changed = False
            for inst in blk.instructions:
                si = getattr(inst, "sync_info", None)
                ow = list(si.on_wait) if si is not None and si.on_wait else []
                if len(ow) > max_waits and inst.engine is not None:
                    for k, wcond in enumerate(ow[:-max_waits]):
                        newl.append(mybir.InstNoOp(
                            name=f"{inst.name}w{k}",
                            engine=inst.engine,
                            sync_info=mybir.SyncInfo(on_wait=[wcond],
                                                     on_update=[]),
                        ))
                    inst.sync_info = mybir.SyncInfo(
                        on_wait=ow[-max_waits:],
                        on_update=list(si.on_update) if si.on_update else [])
                    changed = True
                newl.append(inst)
            if changed:
                blk.instructions = newl


def _emit_body(ctx, tc, x_ap, y_ap, o_ap):
    nc = tc.nc
    n_iter = H // (NSL * G)     # 4
    o_t = o_ap.tensor
    inv_c = 1.0 / C
    yflat = y_ap.rearrange("c h w -> c (h w)")

    xpool = ctx.enter_context(tc.tile_pool(name="xp", bufs=3))
    ypool = ctx.enter_context(tc.tile_pool(name="yp", bufs=3))
    gpool = ctx.enter_context(tc.tile_pool(name="gp", bufs=4))
    ppool = ctx.enter_context(tc.tile_pool(name="pp", bufs=8, space="PSUM"))

    def emit_loads(it):
        h0 = it * NSL * G
        xt = xpool.tile([128, G, W], mybir.dt.float16, name=f"xt{it}", tag="xt")
        yt = ypool.tile([128, YCOLS], mybir.dt.float16, name=f"yt{it}", tag="yt")
        for ss in range(NSL):
            hs = h0 + ss * G
            nc.sync.dma_start(xt[32 * ss:32 * ss + C, :, :],
                              x_ap[:, hs:hs + G, :])
            nc.sync.dma_start(yt[32 * ss:32 * ss + C, LEAD:LEAD + G * W],
                              yflat[:, hs * W:(hs + G) * W])
        nc.vector.memset(yt[:, 0:LEAD], 0.0)
        nc.vector.memset(yt[:, LEAD + G * W:YCOLS], 0.0)
        return xt, yt

    tiles = {0: emit_loads(0)}
    for it in range(n_iter):
        if it + 1 < n_iter:
            tiles[it + 1] = emit_loads(it + 1)
        xt, yt = tiles.pop(it)

        for gp in range(G // GPACK):
            gt = gpool.tile([128, GPACK, NSL, GQ], mybir.dt.float16,
                            name=f"gt{it}_{gp}", tag="gt")
            for gg in range(GPACK):
                g = gp * GPACK + gg
                ps = [ppool.tile([128, GQ], mybir.dt.float32,
                                 name=f"ps{it}_{g}_{ss}", tag="ps",
                                 padded_shape=[128, 512])
                      for ss in range(NSL)]
                done = {ss: 0 for ss in range(NSL)}
                for t, c, ss in MM_SLOTS:
                    base = 32 * ss
                    a = t * 128 + 32 * c
                    nc.tensor.matmul(
                        ps[ss][32 * c:32 * c + 32,
                               t * MMN:(t + 1) * MMN],
                        xt[base:base + C, g, a:a + 32],
                        yt[base:base + C,
                           g * W + a:g * W + a + MMN],
                        start=True, stop=True,
                        tile_position=(base, 32 * c))
                    done[ss] += 1
                    if done[ss] == 16:
                        # emit the copy as soon as this row's psum is full
                        dst = gt[:, gg, ss, :]
                        if ss % 2 == 0:
                            nc.vector.tensor_scalar_mul(dst, ps[ss][:, :],
                                                        inv_c)
                        else:
                            nc.scalar.mul(dst, ps[ss][:, :], inv_c)

            # band-layout output: [gq][p][gg][ss][320] fp16, 10240B/(gq,p)
            gq = it * (G // GPACK) + gp
            dstd = bass.AP(o_t, gq * 128 * GPACK * NSL * GQ,
                           [[GPACK * NSL * GQ, 128], [NSL * GQ, GPACK],
                            [GQ, NSL], [1, GQ]])
            nc.sync.dma_start(dstd, gt[:, :, :, :])


def _build_kernel():
    nc = bass.Bass(trn_type="TRN2", target_bir_lowering=False)
    x_d = nc.dram_tensor("x", [C, H, W], mybir.dt.float16, kind="ExternalInput")
    y_d = nc.dram_tensor("y", [C, H, W], mybir.dt.float16, kind="ExternalInput")
    o_d = nc.dram_tensor("o", [NGQ * 128 * GPACK * NSL * GQ], mybir.dt.float16,
                         kind="ExternalOutput")
    with ExitStack() as ctx:
        tc = ctx.enter_context(tile.TileContext(nc))
        _emit_body(ctx, tc, x_d.ap(), y_d.ap(), o_d.ap())
    _split_waits(nc)
    return nc


_NC_CACHE = None


def _get_nc():
    global _NC_CACHE
    if _NC_CACHE is None:
        _NC_CACHE = _build_kernel()
    return _NC_CACHE


def _unskew(o_flat: np.ndarray) -> np.ndarray:
    """Band layout [gq(16), p(128), gg(4), ss(4), q(320)] fp16 ->
    [D, H, W] fp32 with the w<d triangle zeroed.

    q = t*80 + qq;  p = pc*32 + r;  w = t*128 + pc*32 + r;
    h = it*64 + ss*16 + gp*4 + gg (gq = it*4 + gp);  qq = r + (47 - d).
    """
    R = o_flat.reshape(4, 4, 4, 32, 4, 4, 4, MMN)  # it,gp,pc,r,gg,ss,t,qq
    s = R.strides
    V = np.lib.stride_tricks.as_strided(
        R, shape=(4, 4, 4, 32, 4, 4, 4, D),
        strides=(s[0], s[1], s[2], s[3] + s[7], s[4], s[5], s[6], s[7]))
    X = V.astype(np.float32)  # gather along the fast axis, then view-transpose
    # axes: it,gp,pc,r,gg,ss,t,k -> d(=47-k), h(it,ss,gp,gg), w(t,pc,r)
    Xf = X[..., ::-1]
    out = Xf.transpose(7, 0, 5, 1, 4, 6, 2, 3).reshape(D, H, W)
    out = np.ascontiguousarray(out)
    for d in range(1, D):
        out[d, :, :d] = 0.0
    return out


def kernel(x: np.ndarray, y: np.ndarray, maxdisp=48) -> np.ndarray:
    assert int(maxdisp) == D
    x = np.asarray(x)
    y = np.asarray(y)
    assert x.shape == (B, C, H, W) and y.shape == (B, C, H, W)
    x16 = np.ascontiguousarray(x.astype(np.float16))
    y16 = np.ascontiguousarray(y.astype(np.float16))

    nc = _get_nc()
    in_maps = [{"x": x16[b], "y": y16[b]} for b in range(B)]
    res = bass_utils.run_bass_kernel_spmd(nc, in_maps, core_ids=list(range(B)))

    out = np.empty((B, D, H, W), dtype=np.float32)
    for b in range(B):
        out[b] = _unskew(np.asarray(res.results[b]["o"]))
    return out


if __name__ == "__main__":
    rng = np.random.default_rng(0)
    x = rng.standard_normal((B, C, H, W), dtype=np.float32)
    y = rng.standard_normal((B, C, H, W), dtype=np.float32)
    out = kernel(x=x, y=y, maxdisp=D)
    print("kernel output:", out.shape, out.dtype)


# revision 14
# speedup vs baseline: 1.3382x; 1.3382x over previous
"""Trainium2 Bass kernel for nn_Correlation (stereo cost volume).

  out[b, d, h, w] = mean_c( x[b,c,h,w] * y[b,c,h,w-d] ),  w >= d else 0
  B=8, C=32, H=256, W=512, D=48  (maxdisp=48)

Sharding: data-parallel over batch B across the 8 NeuronCores (one batch
element per core).  Each core computes its full [D, H, W] cost volume.

Per-core algorithm (fp16 inputs, fp32 psum, fp16 output):
  - Host pre-casts x/y to fp16 (halves input HBM traffic; PE runs fp16 at
    full rate and accumulates fp32, so rel-err stays ~1e-3).
  - x/y are staged in FOUR 32-partition slabs (4 consecutive h-row groups
    at partitions 0/32/64/96).  y rows are contiguous per slab with a
    47-col lead + 48-col tail (memset once per tile) so every matmul
    window is in-bounds; out-of-range products land on band cells with
    d<0 or d>w, which are never extracted / zeroed on the host.
  - Per (h, 128-col w-tile): TWO M=64 col-tiled matmuls (c = 0..1):
      stationary x[w0+64c : w0+64c+64] -> psum parts 64c:64c+64
      moving    y[w0+64c-47 .. w0+64c+65)   (K=32, N=112)
    The 48 valid outputs of partition j=p%64 land on diagonal q = j..j+47
    of its own 112-col block, so a full h-row's valid band is ONE dense
    [128, 448] psum bank (vs [128, 700] full-window).  Emission rotates
    (row-group, col-group) so LDWEIGHTS overlaps matmuls.
  - One DVE/ACT copy per h-row scales the [128,448] psum by 1/32 into a
    fp16 G tile [128, 2, 4, 448] (2 h-groups x 4 slab-rows), DMA'd
    straight to the DRAM output in band layout (7168B-contiguous
    descriptors).  No DRAM->DRAM skew pass, no scratch round trip.
  - Input loads for iteration it+1 are emitted before it's compute
    (bufs=3) so they stream during compute instead of serializing at
    iteration boundaries on the FIFO sync queue.
  - The host un-skews the diagonals with a strided numpy view, zeroes
    the w<d triangle, and assembles [B, D, H, W] fp32.
"""

import sys

sys.path.insert(0, "/opt/trn_rl_repo")

import numpy as np
from contextlib import ExitStack

import concourse.bass as bass
import concourse.tile as tile
from concourse import mybir
from concourse import bass_utils

B = 8
C = 32
H = 256
W = 512
D = 48
LEAD = D - 1            # 47 cols before row 0 of each slab
TAIL = 48               # cols after the last row (t=3,c=1 window overrun)
NW = W // 128           # 4 w-tiles per row
MMN = 112               # moving cols per col-tile matmul (64 + 47, padded)
GQ = NW * MMN           # 448 band cols per h-row
G = 16                  # h rows per slab per iteration
NSL = 4                 # slabs (row-groups) per iteration
GPACK = 2               # g-iterations packed per output DMA
NGQ = H // (NSL * GPACK)  # 32 output groups (8 h-rows each)
YCOLS = LEAD + G * W + TAIL

# (c, ss) emission order: consecutive matmuls differ in both row-group
# and col-group so LDWEIGHTS and PSUM drains overlap across subarrays
MM_ORDER = [(0, 0), (1, 1), (0, 2), (1, 3), (1, 0), (0, 1), (1, 2), (0, 3)]


def _split_waits(nc, max_waits=1):
    """Walrus codegen accepts at most ONE sync wait per instruction; Tile
    attaches several.  Split extra waits onto preceding NoOps on the same
    engine queue (dispatch is in-order, waits gate dispatch)."""
    for fn in nc.m.functions:
        for blk in fn.blocks:
            newl = []
            changed = False
            for inst in blk.instructions:
                si = getattr(inst, "sync_info", None)
                ow = list(si.on_wait) if si is not None and si.on_wait else []
                if len(ow) > max_waits and inst.engine is not None:
                    for k, wcond in enumerate(ow[:-max_waits]):
                        newl.append(mybir.InstNoOp(
                            name=f"{inst.name}w{k}",
                            engine=inst.engine,
                            sync_info=mybir.SyncInfo(on_wait=[wcond],
                                                     on_update=[]),
                        ))
                    inst.sync_info = mybir.SyncInfo(
                        on_wait=ow[-max_waits:],
                        on_update=list(si.on_update) if si.on_update else [])
                    changed = True
                newl.append(inst)
            if changed:
                blk.instructions = newl


def _emit_body(ctx, tc, x_ap, y_ap, o_ap):
    nc = tc.nc
    n_iter = H // (NSL * G)     # 4
    o_t = o_ap.tensor
    inv_c = 1.0 / C
    yflat = y_ap.rearrange("c h w -> c (h w)")

    xpool = ctx.enter_context(tc.tile_pool(name="xp", bufs=3))
    ypool = ctx.enter_context(tc.tile_pool(name="yp", bufs=3))
    gpool = ctx.enter_context(tc.tile_pool(name="gp", bufs=6))
    ppool = ctx.enter_context(tc.tile_pool(name="pp", bufs=8, space="PSUM"))

    def emit_loads(it):
        h0 = it * NSL * G
        xt = xpool.tile([128, G, W], mybir.dt.float16, name=f"xt{it}", tag="xt")
        yt = ypool.tile([128, YCOLS], mybir.dt.float16, name=f"yt{it}", tag="yt")
        for ss in range(NSL):
            hs = h0 + ss * G
            # input loads go on the ACT HWDGE ring so they never queue
            # ahead of output DMAs in the SP ring FIFO at it boundaries
            nc.scalar.dma_start(xt[32 * ss:32 * ss + C, :, :],
                                x_ap[:, hs:hs + G, :])
            nc.scalar.dma_start(yt[32 * ss:32 * ss + C, LEAD:LEAD + G * W],
                                yflat[:, hs * W:(hs + G) * W])
        nc.vector.memset(yt[:, 0:LEAD], 0.0)
        nc.vector.memset(yt[:, LEAD + G * W:YCOLS], 0.0)
        return xt, yt

    tiles = {0: emit_loads(0)}
    for it in range(n_iter):
        if it + 1 < n_iter:
            tiles[it + 1] = emit_loads(it + 1)
        xt, yt = tiles.pop(it)

        for gp in range(G // GPACK):
            gt = gpool.tile([128, GPACK, NSL, GQ], mybir.dt.float16,
                            name=f"gt{it}_{gp}", tag="gt")
            for gg in range(GPACK):
                g = gp * GPACK + gg
                ps = [ppool.tile([128, GQ], mybir.dt.float32,
                                 name=f"ps{it}_{g}_{ss}", tag="ps",
                                 padded_shape=[128, 512])
                      for ss in range(NSL)]
                for t in range(NW):
                    w0 = t * 128
                    for c, ss in MM_ORDER:
                        base = 32 * ss
                        a = w0 + 64 * c
                        nc.tensor.matmul(
                            ps[ss][64 * c:64 * c + 64,
                                   t * MMN:(t + 1) * MMN],
                            xt[base:base + C, g, a:a + 64],
                            yt[base:base + C,
                               g * W + a:g * W + a + MMN],
                            start=True, stop=True,
                            tile_position=(base, 64 * c))
                for ss in range(NSL):
                    dst = gt[:, gg, ss, :]
                    if ss % 2 == 0:
                        nc.vector.tensor_scalar_mul(dst, ps[ss][:, :], inv_c)
                    else:
                        nc.scalar.mul(dst, ps[ss][:, :], inv_c)

            # band-layout output: [gq][p][gg][ss][448] fp16, 7168B/(gq,p)
            gq = it * (G // GPACK) + gp
            dstd = bass.AP(o_t, gq * 128 * GPACK * NSL * GQ,
                           [[GPACK * NSL * GQ, 128], [NSL * GQ, GPACK],
                            [GQ, NSL], [1, GQ]])
            nc.sync.dma_start(dstd, gt[:, :, :, :])


def _build_kernel():
    nc = bass.Bass(trn_type="TRN2", target_bir_lowering=False)
    x_d = nc.dram_tensor("x", [C, H, W], mybir.dt.float16, kind="ExternalInput")
    y_d = nc.dram_tensor("y", [C, H, W], mybir.dt.float16, kind="ExternalInput")
    o_d = nc.dram_tensor("o", [NGQ * 128 * GPACK * NSL * GQ], mybir.dt.float16,
                         kind="ExternalOutput")
    with ExitStack() as ctx:
        tc = ctx.enter_context(tile.TileContext(nc))
        _emit_body(ctx, tc, x_d.ap(), y_d.ap(), o_d.ap())
    _split_waits(nc)
    return nc


_NC_CACHE = None


def _get_nc():
    global _NC_CACHE
    if _NC_CACHE is None:
        _NC_CACHE = _build_kernel()
    return _NC_CACHE


def _unskew(o_flat: np.ndarray) -> np.ndarray:
    """Band layout [gq(32), p(128), gg(2), ss(4), q(448)] fp16 ->
    [D, H, W] fp32 with the w<d triangle zeroed.

    q = t*112 + qq;  p = pc*64 + r;  w = t*128 + pc*64 + r;
    h = it*64 + ss*16 + gp*2 + gg (gq = it*8 + gp);  qq = r + (47 - d).
    """
    R = o_flat.reshape(4, 8, 2, 64, 2, 4, 4, MMN)  # it,gp,pc,r,gg,ss,t,qq
    s = R.strides
    V = np.lib.stride_tricks.as_strided(
        R, shape=(4, 8, 2, 64, 2, 4, 4, D),
        strides=(s[0], s[1], s[2], s[3] + s[7], s[4], s[5], s[6], s[7]))
    X = V.astype(np.float32)  # gather along the fast axis, then view-transpose
    # axes: it,gp,pc,r,gg,ss,t,k -> d(=47-k), h(it,ss,gp,gg), w(t,pc,r)
    Xf = X[..., ::-1]
    out = Xf.transpose(7, 0, 5, 1, 4, 6, 2, 3).reshape(D, H, W)
    out = np.ascontiguousarray(out)
    for d in range(1, D):
        out[d, :, :d] = 0.0
    return out


def kernel(x: np.ndarray, y: np.ndarray, maxdisp=48) -> np.ndarray:
    assert int(maxdisp) == D
    x = np.asarray(x)
    y = np.asarray(y)
    assert x.shape == (B, C, H, W) and y.shape == (B, C, H, W)
    x16 = np.ascontiguousarray(x.astype(np.float16))
    y16 = np.ascontiguousarray(y.astype(np.float16))

    nc = _get_nc()
    in_maps = [{"x": x16[b], "y": y16[b]} for b in range(B)]
    res = bass_utils.run_bass_kernel_spmd(nc, in_maps, core_ids=list(range(B)))

    out = np.empty((B, D, H, W), dtype=np.float32)
    for b in range(B):
        out[b] = _unskew(np.asarray(res.results[b]["o"]))
    return out


if __name__ == "__main__":
    rng = np.random.default_rng(0)
    x = rng.standard_normal((B, C, H, W), dtype=np.float32)
    y = rng.standard_normal((B, C, H, W), dtype=np.float32)
    out = kernel(x=x, y=y, maxdisp=D)
    print("kernel output:", out.shape, out.dtype)
